# revision 16
# baseline (speedup 1.0000x reference)
"""Trainium2 Bass kernel for quantized ConvBNReLU1D (pointwise conv k=1).

Reference computation (see problem spec):
    wq  = fake_quant_int8(W)  (per-tensor power-of-two scale)
    bq  = fake_quant_int8(b)
    y   = wq @ x + bq                  # [Cout,Cin] x [B,Cin,N]
    y   = y * inv + (beta - mean*inv)  # BN inference, inv = gamma*rsqrt(var+eps)
    y   = clip(round(relu(y)/as), 0, 255) * as   # QuantReLU

Strategy (v3 — minimize HBM bytes, then pack the DMA stream):
  - Data-parallel over batch: 32 batches -> 4 per core on 8 cores.
  - Host precomputes the per-channel constants (wq/bq fake-quant is
    bitwise-identical to the fp32 reference; BN+act_scale folded) so the
    device epilogue is one ScalarE ACTIVATE per tile.
  - x is sent as plain bf16 (half the bytes of fp32). wq is exactly
    representable in bf16 (8-bit integer x power of two), so the only
    error is bf16 rounding of x: measured rel err 0.0039 (max one quant
    step), same as the baseline's fp32-split pipeline.
  - Output goes to HBM as u8 quantization codes (the result has only 256
    distinct values: u8 * act_scale); dequant happens on host during
    unshard. 1 byte/elem instead of 4.
  - v4 pipeline fixes over v2 (63 us measured):
      * all constants packed into TWO front-loaded DMAs at the head of
        the scalar ring (v2's 8 strided const loads issued so slowly the
        first matmul waited until +17 us);
      * x buffered at full depth (all 8 tiles) and output tiles 4-deep:
        no WAR stalls in the DMA stream (v2 stalled ~4 us mid-run);
      * transfers stay full-width [128, 4096] — v3 measured that halving
        them costs ~30% DMA efficiency (strided vs contiguous streams).
  - DMA per core: in 8.4 MB (bf16) + out 4.2 MB (u8) = 12.6 MB ~= 35 us
    at the ~358 GB/s HBM/core roofline; TensorE ~28 us; ScalarE ~32 us;
    VectorE/GpSimd unused. ~5-8 us fixed runtime preamble on top.
"""

import os
import sys

import numpy as np

for _p in ("/opt/trn_rl_repo", "/root/.axon_site/_ro/trn_rl_repo"):
    if os.path.isdir(_p) and _p not in sys.path:
        sys.path.insert(0, _p)

from contextlib import ExitStack

import ml_dtypes

import concourse.bacc as bacc
import concourse.tile as tile
from concourse import mybir
from concourse.bass import ts
from concourse.bass_utils import run_bass_kernel_spmd

F32 = mybir.dt.float32
BF16 = mybir.dt.bfloat16
U8 = mybir.dt.uint8
AF = mybir.ActivationFunctionType
ALU = mybir.AluOpType

N_CORES = 8
B, CIN, COUT, N = 32, 256, 256, 4096
B_SH = B // N_CORES  # batches per core
NTILE = 512          # matmul free dim (one fp32 PSUM bank)
EP_BANKS = 4         # PSUM banks per epilogue tile (ACT width = 512*EP_BANKS)
EPW = NTILE * EP_BANKS
NEP = N // EPW       # epilogue tiles per row block (= x half-tiles)
KC = CIN // 128      # K chunks
MC = COUT // 128     # output-channel chunks

QMAX_W = 127.0
BN_EPS = 1e-5

_NC_CACHE = []
LAST_RESULTS = None  # BassKernelResults of the last run (for profiling)


def _build_nc():
    nc = bacc.Bacc("TRN2", target_bir_lowering=False)
    xh_s = nc.declare_dram_parameter("xh_s", [B_SH, CIN, N], BF16, isOutput=False)
    # all 4 lhsT chunks packed side by side: col block k*MC+mo is
    # wT[k*128:(k+1)*128, mo*128:(mo+1)*128]; split in partition halves so
    # each HWDGE ring generates 64 descriptors (~1.6 us) in parallel
    w_top = nc.declare_dram_parameter("w_top", [64, KC * MC * 128], BF16, isOutput=False)
    w_bot = nc.declare_dram_parameter("w_bot", [64, KC * MC * 128], BF16, isOutput=False)
    # per-channel vectors packed: col mo = sv chunk, col MC+mo = bv chunk
    vec_all = nc.declare_dram_parameter("vec_all", [128, 2 * MC], F32, isOutput=False)
    y_s = nc.declare_dram_parameter("y_s", [B_SH, COUT, N], U8, isOutput=True)

    with ExitStack() as ctx:
        tc = ctx.enter_context(tile.TileContext(nc))
        consts = ctx.enter_context(tc.tile_pool(name="consts", bufs=1))
        xpool = ctx.enter_context(tc.tile_pool(name="xpool", bufs=B_SH * KC))
        opool = ctx.enter_context(tc.tile_pool(name="opool", bufs=4))
        tmppool = ctx.enter_context(tc.tile_pool(name="tmppool", bufs=2))
        pspool = ctx.enter_context(
            tc.tile_pool(name="pspool", bufs=8 // EP_BANKS, space="PSUM")
        )

        # The schedule is limited by HWDGE descriptor generation (~25 ns
        # per descriptor = one per partition line, serial per ring), and
        # only the two HWDGE rings complete transfers quickly (gpsimd's
        # SWDGE completions measured ~7 us late — never put latency-
        # critical loads there). So the early generation slots go to what
        # the pipeline ramp needs, split into 64-partition halves that the
        # two rings generate in parallel:
        #   sync:   w_top, x0k0_top, x0k1_top, then batches 1-3 full
        #   scalar: w_bot, x0k0_bot, x0k1_bot, vec, then the 8 stores
        w_sb = consts.tile([128, KC * MC * 128], BF16, tag="w")
        nc.sync.dma_start(out=w_sb[:64, :], in_=w_top[:, :])
        nc.scalar.dma_start(out=w_sb[64:, :], in_=w_bot[:, :])

        # Warm the PE clock (HAM un-throttles only after sustained matmul
        # activity; cold-start matmuls run at 1/2-1/4 rate): dummy matmuls
        # off the weight tile, into a PSUM tile that cycles back into the
        # real rotation, keep the array busy while the first x tiles are
        # still in flight.
        warm = pspool.tile([128, EPW], F32, tag="ps")
        for _ in range(10):
            nc.tensor.matmul(
                warm[:, ts(0, NTILE)], lhsT=w_sb[:64, ts(0, 128)],
                rhs=w_sb[:64, :NTILE], start=True, stop=True,
            )

        ep_i = 0  # global epilogue tile index (Scalar/Vector split)
        vec_sb = None
        for b in range(B_SH):
            # Full [128, 4096] x tiles: 8 KB contiguous per partition —
            # splitting transfers in the free dim costs ~30% DMA efficiency
            # (measured in a prior rev). Batch 0 is the exception: its two
            # tiles load as 64-partition halves, one half-pair per HWDGE
            # ring, so both land ~4 us sooner and compute starts early.
            xh_k = []
            for k in range(KC):
                t = xpool.tile([128, N], BF16, tag="x")
                if b == 0:
                    nc.sync.dma_start(
                        out=t[:64, :], in_=xh_s[b, k * 128 : k * 128 + 64, :]
                    )
                    nc.scalar.dma_start(
                        out=t[64:, :], in_=xh_s[b, k * 128 + 64 : (k + 1) * 128, :]
                    )
                else:
                    nc.sync.dma_start(
                        out=t, in_=xh_s[b, k * 128 : (k + 1) * 128, :]
                    )
                xh_k.append(t)
            if b == 0:
                # vec is needed first by the first epilogue tile (~+15);
                # it queues on the scalar ring after batch 0's x halves.
                vec_sb = consts.tile([128, 2 * MC], F32, tag="vec")
                nc.scalar.dma_start(out=vec_sb, in_=vec_all[:, :])
            for mo in range(MC):
                ot = opool.tile([128, N], U8, tag="o")
                for ne in range(NEP):
                    ps = pspool.tile([128, EPW], F32, tag="ps")
                    # k-major: x's k1 tile is not needed until four matmuls
                    # after k0, giving its in-flight load extra slack
                    for k in range(KC):
                        for sb in range(EP_BANKS):
                            nt = ne * EP_BANKS + sb
                            nc.tensor.matmul(
                                ps[:, ts(sb, NTILE)],
                                lhsT=w_sb[:, ts(k * MC + mo, 128)],
                                rhs=xh_k[k][:, ts(nt, NTILE)],
                                start=(k == 0), stop=(k == KC - 1),
                            )
                    # Epilogue u8 = clip(round(psum*sv + bv), 0, 255).
                    # ScalarE does 11 tiles (one ACTIVATE each: relu+affine+
                    # sat-u8 convert, exact round-half-even); the otherwise-
                    # idle VectorE takes 5 (affine into f32 tmp — PSUM reads
                    # are 1x on DVE — then clamp to [0,255] and convert;
                    # clamp-first == round-first here, and safe even if the
                    # DVE u8 convert wrapped; measured bit-compatible to
                    # 3 elements in 33.5M). Neither engine paces Tensor.
                    if ep_i % 3 != 1:
                        nc.scalar.activation(
                            ot[:, ts(ne, EPW)], ps, AF.Relu,
                            bias=vec_sb[:, MC + mo : MC + mo + 1],
                            scale=vec_sb[:, mo : mo + 1],
                        )
                    else:
                        tmp = tmppool.tile([128, EPW], F32, tag="tmp")
                        nc.vector.tensor_scalar(
                            tmp, ps,
                            vec_sb[:, mo : mo + 1],
                            vec_sb[:, MC + mo : MC + mo + 1],
                            ALU.mult, ALU.add,
                        )
                        nc.vector.tensor_scalar(
                            ot[:, ts(ne, EPW)], tmp, 255.0, 0.0,
                            ALU.min, ALU.max,
                        )
                    ep_i += 1
                if b == B_SH - 1 and mo == MC - 1:
                    # Last tile: store as two halves on two rings so the
                    # post-compute tail pays one parallel descriptor-gen
                    # (~3.2 us) plus a half-size transfer.
                    nc.sync.dma_start(
                        out=y_s[b, mo * 128 : (mo + 1) * 128, : N // 2],
                        in_=ot[:, : N // 2],
                    )
                    nc.scalar.dma_start(
                        out=y_s[b, mo * 128 : (mo + 1) * 128, N // 2 :],
                        in_=ot[:, N // 2 :],
                    )
                else:
                    nc.scalar.dma_start(
                        out=y_s[b, mo * 128 : (mo + 1) * 128, :], in_=ot
                    )
    nc.compile()
    return nc


def _host_fold(W, b, gamma, beta, running_mean, running_var, act_scale):
    """Fake-quant W/b exactly as the fp32 reference, fold BN + act scale."""
    f32 = np.float32

    def po2_scale(t):
        maxabs = np.maximum(np.max(np.abs(t)), f32(1e-12)).astype(f32)
        # log2/ceil/exp2 of an f32 value; result is an exact power of two.
        return np.exp2(np.ceil(np.log2(maxabs / f32(QMAX_W)))).astype(f32)

    def fake_quant(t, s):
        return (np.clip(np.round(t / s), -128.0, 127.0) * s).astype(f32)

    wq = fake_quant(W.astype(f32), po2_scale(W.astype(f32)))
    bq = fake_quant(b.astype(f32), po2_scale(b.astype(f32)))
    inv = (gamma.astype(f32) / np.sqrt(running_var.astype(f32) + f32(BN_EPS))).astype(f32)
    shift = (beta.astype(f32) - running_mean.astype(f32) * inv).astype(f32)
    a_s = f32(act_scale)
    sv = (inv / a_s).astype(f32)                    # per-channel matmul scale
    bv = ((bq * inv + shift) / a_s).astype(f32)     # per-channel bias
    # wq is an 8-bit integer times a power of two -> exact in bf16
    wT = np.ascontiguousarray(wq.T)                 # [Cin, Cout] f32
    w_pack = np.empty((128, KC * MC * 128), dtype=ml_dtypes.bfloat16)
    for k in range(KC):
        for mo in range(MC):
            j = (k * MC + mo) * 128
            w_pack[:, j : j + 128] = wT[
                k * 128 : (k + 1) * 128, mo * 128 : (mo + 1) * 128
            ].astype(ml_dtypes.bfloat16)
    w_pack = np.ascontiguousarray(w_pack)
    vec_pack = np.empty((128, 2 * MC), dtype=np.float32)
    for mo in range(MC):
        vec_pack[:, mo] = sv[mo * 128 : (mo + 1) * 128]
        vec_pack[:, MC + mo] = bv[mo * 128 : (mo + 1) * 128]
    return w_pack, vec_pack, a_s


def kernel(x, W, b, gamma, beta, running_mean, running_var, act_scale):
    global LAST_RESULTS
    if not _NC_CACHE:
        _NC_CACHE.append(_build_nc())
    nc = _NC_CACHE[0]

    w_pack, vec_pack, a_s = _host_fold(
        W, b, gamma, beta, running_mean, running_var, act_scale
    )
    x_hi = np.ascontiguousarray(x, dtype=np.float32).astype(ml_dtypes.bfloat16)

    w_top = np.ascontiguousarray(w_pack[:64])
    w_bot = np.ascontiguousarray(w_pack[64:])
    in_maps = []
    for c in range(N_CORES):
        sl = slice(c * B_SH, (c + 1) * B_SH)
        in_maps.append(
            {"xh_s": x_hi[sl], "w_top": w_top, "w_bot": w_bot, "vec_all": vec_pack}
        )

    trace = bool(os.environ.get("KERNEL_TRACE"))
    try:
        res = run_bass_kernel_spmd(
            nc, in_maps, core_ids=list(range(N_CORES)), trace=trace
        )
    except Exception:
        if not trace:
            raise
        # trace path unavailable (e.g. NTFF hook missing) — run untraced
        res = run_bass_kernel_spmd(
            nc, in_maps, core_ids=list(range(N_CORES)), trace=False
        )
    LAST_RESULTS = res
    codes = np.concatenate([r["y_s"] for r in res.results], axis=0)
    # dequantize the u8 codes during unshard: y = codes * act_scale
    lut = (np.arange(256, dtype=np.float32) * a_s).astype(np.float32)
    return lut[codes]


# revision 21
# speedup vs baseline: 1.0153x; 1.0153x over previous
"""Trainium2 Bass kernel for quantized ConvBNReLU1D (pointwise conv k=1).

Reference computation (see problem spec):
    wq  = fake_quant_int8(W)  (per-tensor power-of-two scale)
    bq  = fake_quant_int8(b)
    y   = wq @ x + bq                  # [Cout,Cin] x [B,Cin,N]
    y   = y * inv + (beta - mean*inv)  # BN inference, inv = gamma*rsqrt(var+eps)
    y   = clip(round(relu(y)/as), 0, 255) * as   # QuantReLU

Strategy (v3 — minimize HBM bytes, then pack the DMA stream):
  - Data-parallel over batch: 32 batches -> 4 per core on 8 cores.
  - Host precomputes the per-channel constants (wq/bq fake-quant is
    bitwise-identical to the fp32 reference; BN+act_scale folded) so the
    device epilogue is one ScalarE ACTIVATE per tile.
  - x is sent as plain bf16 (half the bytes of fp32). wq is exactly
    representable in bf16 (8-bit integer x power of two), so the only
    error is bf16 rounding of x: measured rel err 0.0039 (max one quant
    step), same as the baseline's fp32-split pipeline.
  - Output goes to HBM as u8 quantization codes (the result has only 256
    distinct values: u8 * act_scale); dequant happens on host during
    unshard. 1 byte/elem instead of 4.
  - v4 pipeline fixes over v2 (63 us measured):
      * all constants packed into TWO front-loaded DMAs at the head of
        the scalar ring (v2's 8 strided const loads issued so slowly the
        first matmul waited until +17 us);
      * x buffered at full depth (all 8 tiles) and output tiles 4-deep:
        no WAR stalls in the DMA stream (v2 stalled ~4 us mid-run);
      * transfers stay full-width [128, 4096] — v3 measured that halving
        them costs ~30% DMA efficiency (strided vs contiguous streams).
  - DMA per core: in 8.4 MB (bf16) + out 4.2 MB (u8) = 12.6 MB ~= 35 us
    at the ~358 GB/s HBM/core roofline; TensorE ~28 us; ScalarE ~32 us;
    VectorE/GpSimd unused. ~5-8 us fixed runtime preamble on top.
"""

import os
import sys

import numpy as np

for _p in ("/opt/trn_rl_repo", "/root/.axon_site/_ro/trn_rl_repo"):
    if os.path.isdir(_p) and _p not in sys.path:
        sys.path.insert(0, _p)

from contextlib import ExitStack

import ml_dtypes

import concourse.bacc as bacc
import concourse.tile as tile
from concourse import mybir
from concourse.bass import ts
from concourse.bass_utils import run_bass_kernel_spmd

F32 = mybir.dt.float32
BF16 = mybir.dt.bfloat16
U8 = mybir.dt.uint8
AF = mybir.ActivationFunctionType
ALU = mybir.AluOpType

N_CORES = 8
B, CIN, COUT, N = 32, 256, 256, 4096
B_SH = B // N_CORES  # batches per core
NTILE = 512          # matmul free dim (one fp32 PSUM bank)
EP_BANKS = 4         # PSUM banks per epilogue tile (ACT width = 512*EP_BANKS)
EPW = NTILE * EP_BANKS
NEP = N // EPW       # epilogue tiles per row block (= x half-tiles)
KC = CIN // 128      # K chunks
MC = COUT // 128     # output-channel chunks

QMAX_W = 127.0
BN_EPS = 1e-5

_NC_CACHE = []
LAST_RESULTS = None  # BassKernelResults of the last run (for profiling)


def _build_nc():
    nc = bacc.Bacc("TRN2", target_bir_lowering=False)
    xh_s = nc.declare_dram_parameter("xh_s", [B_SH, CIN, N], BF16, isOutput=False)
    # all 4 lhsT chunks packed side by side: col block k*MC+mo is
    # wT[k*128:(k+1)*128, mo*128:(mo+1)*128]
    w_all = nc.declare_dram_parameter("w_all", [128, KC * MC * 128], BF16, isOutput=False)
    # per-channel vectors packed: col mo = sv chunk, col MC+mo = bv chunk
    vec_all = nc.declare_dram_parameter("vec_all", [128, 2 * MC], F32, isOutput=False)
    y_s = nc.declare_dram_parameter("y_s", [B_SH, COUT, N], U8, isOutput=True)

    with ExitStack() as ctx:
        tc = ctx.enter_context(tile.TileContext(nc))
        consts = ctx.enter_context(tc.tile_pool(name="consts", bufs=1))
        xpool = ctx.enter_context(tc.tile_pool(name="xpool", bufs=B_SH * KC))
        opool = ctx.enter_context(tc.tile_pool(name="opool", bufs=6))
        tmppool = ctx.enter_context(tc.tile_pool(name="tmppool", bufs=2))
        pspool = ctx.enter_context(
            tc.tile_pool(name="pspool", bufs=8 // EP_BANKS, space="PSUM")
        )

        # The schedule is limited by HWDGE descriptor generation (~25 ns
        # per descriptor = one per partition line, serial per ring) and by
        # first-transfer latency (~4.5 us issue-to-land for a 1 MB tile).
        # Only the two HWDGE rings complete transfers promptly (gpsimd's
        # SWDGE completions measured ~7 us lazy). Ring assignment by
        # latency-criticality:
        #   sync:   x loads for batches 0-2 (x0k0 in the very first slot);
        #   scalar: w (warmups + all matmuls key on it), vec, batch-3 x,
        #           and the final stores;
        #   gpsimd: all non-final stores (latency-tolerant, keeps their
        #           descriptor generation off both HWDGE rings).
        w_sb = consts.tile([128, KC * MC * 128], BF16, tag="w")
        nc.scalar.dma_start(out=w_sb, in_=w_all[:, :])
        vec_sb = consts.tile([128, 2 * MC], F32, tag="vec")
        nc.scalar.dma_start(out=vec_sb, in_=vec_all[:, :])

        # Warm the PE clock (HAM un-throttles only after sustained matmul
        # activity; cold-start matmuls run at 1/2-1/4 rate): dummy matmuls
        # off the weight tile, into a PSUM tile that cycles back into the
        # real rotation, keep the array busy while the first x tiles are
        # still in flight.
        warm = pspool.tile([128, EPW], F32, tag="ps")
        for _ in range(12):
            nc.tensor.matmul(
                warm[:, ts(0, NTILE)], lhsT=w_sb[:, ts(0, 128)],
                rhs=w_sb[:, :NTILE], start=True, stop=True,
            )

        ep_i = 0  # global epilogue tile index (Scalar/Vector split)
        for b in range(B_SH):
            # Full [128, 4096] x tiles: 8 KB contiguous per partition —
            # splitting transfers (free-dim or partition-dim) measurably
            # costs more in DMA efficiency than the earlier ramp it buys.
            xh_k = []
            for k in range(KC):
                t = xpool.tile([128, N], BF16, tag="x")
                ring = nc.scalar if b == B_SH - 1 else nc.sync
                ring.dma_start(out=t, in_=xh_s[b, k * 128 : (k + 1) * 128, :])
                xh_k.append(t)
            for mo in range(MC):
                ot = opool.tile([128, N], U8, tag="o")
                for ne in range(NEP):
                    ps = pspool.tile([128, EPW], F32, tag="ps")
                    # k-major: x's k1 tile is not needed until four matmuls
                    # after k0, giving its in-flight load extra slack
                    for k in range(KC):
                        for sb in range(EP_BANKS):
                            nt = ne * EP_BANKS + sb
                            nc.tensor.matmul(
                                ps[:, ts(sb, NTILE)],
                                lhsT=w_sb[:, ts(k * MC + mo, 128)],
                                rhs=xh_k[k][:, ts(nt, NTILE)],
                                start=(k == 0), stop=(k == KC - 1),
                            )
                    # Epilogue u8 = clip(round(psum*sv + bv), 0, 255).
                    # ScalarE does 11 tiles (one ACTIVATE each: relu+affine+
                    # sat-u8 convert, exact round-half-even); the otherwise-
                    # idle VectorE takes 5 (affine into f32 tmp — PSUM reads
                    # are 1x on DVE — then clamp to [0,255] and convert;
                    # clamp-first == round-first here, and safe even if the
                    # DVE u8 convert wrapped; measured bit-compatible to
                    # 3 elements in 33.5M). Neither engine paces Tensor.
                    if ep_i % 3 != 1:
                        nc.scalar.activation(
                            ot[:, ts(ne, EPW)], ps, AF.Relu,
                            bias=vec_sb[:, MC + mo : MC + mo + 1],
                            scale=vec_sb[:, mo : mo + 1],
                        )
                    else:
                        tmp = tmppool.tile([128, EPW], F32, tag="tmp")
                        nc.vector.tensor_scalar(
                            tmp, ps,
                            vec_sb[:, mo : mo + 1],
                            vec_sb[:, MC + mo : MC + mo + 1],
                            ALU.mult, ALU.add,
                        )
                        nc.vector.tensor_scalar(
                            ot[:, ts(ne, EPW)], tmp, 255.0, 0.0,
                            ALU.min, ALU.max,
                        )
                    ep_i += 1
                if b == B_SH - 1 and mo == MC - 1:
                    # Last tile: store as two halves on the two HWDGE rings
                    # (both idle by now) so the post-compute tail pays one
                    # parallel descriptor-gen plus a half-size transfer.
                    nc.sync.dma_start(
                        out=y_s[b, mo * 128 : (mo + 1) * 128, : N // 2],
                        in_=ot[:, : N // 2],
                    )
                    nc.scalar.dma_start(
                        out=y_s[b, mo * 128 : (mo + 1) * 128, N // 2 :],
                        in_=ot[:, N // 2 :],
                    )
                elif b == B_SH - 1:
                    nc.scalar.dma_start(
                        out=y_s[b, mo * 128 : (mo + 1) * 128, :], in_=ot
                    )
                else:
                    # Early/mid stores are latency-tolerant: put them on the
                    # gpsimd SWDGE ring to keep both HWDGE generators free
                    # for the latency-critical load stream.
                    nc.gpsimd.dma_start(
                        out=y_s[b, mo * 128 : (mo + 1) * 128, :], in_=ot
                    )
    nc.compile()
    return nc


def _host_fold(W, b, gamma, beta, running_mean, running_var, act_scale):
    """Fake-quant W/b exactly as the fp32 reference, fold BN + act scale."""
    f32 = np.float32

    def po2_scale(t):
        maxabs = np.maximum(np.max(np.abs(t)), f32(1e-12)).astype(f32)
        # log2/ceil/exp2 of an f32 value; result is an exact power of two.
        return np.exp2(np.ceil(np.log2(maxabs / f32(QMAX_W)))).astype(f32)

    def fake_quant(t, s):
        return (np.clip(np.round(t / s), -128.0, 127.0) * s).astype(f32)

    wq = fake_quant(W.astype(f32), po2_scale(W.astype(f32)))
    bq = fake_quant(b.astype(f32), po2_scale(b.astype(f32)))
    inv = (gamma.astype(f32) / np.sqrt(running_var.astype(f32) + f32(BN_EPS))).astype(f32)
    shift = (beta.astype(f32) - running_mean.astype(f32) * inv).astype(f32)
    a_s = f32(act_scale)
    sv = (inv / a_s).astype(f32)                    # per-channel matmul scale
    bv = ((bq * inv + shift) / a_s).astype(f32)     # per-channel bias
    # wq is an 8-bit integer times a power of two -> exact in bf16
    wT = np.ascontiguousarray(wq.T)                 # [Cin, Cout] f32
    w_pack = np.empty((128, KC * MC * 128), dtype=ml_dtypes.bfloat16)
    for k in range(KC):
        for mo in range(MC):
            j = (k * MC + mo) * 128
            w_pack[:, j : j + 128] = wT[
                k * 128 : (k + 1) * 128, mo * 128 : (mo + 1) * 128
            ].astype(ml_dtypes.bfloat16)
    w_pack = np.ascontiguousarray(w_pack)
    vec_pack = np.empty((128, 2 * MC), dtype=np.float32)
    for mo in range(MC):
        vec_pack[:, mo] = sv[mo * 128 : (mo + 1) * 128]
        vec_pack[:, MC + mo] = bv[mo * 128 : (mo + 1) * 128]
    return w_pack, vec_pack, a_s


def kernel(x, W, b, gamma, beta, running_mean, running_var, act_scale):
    global LAST_RESULTS
    if not _NC_CACHE:
        _NC_CACHE.append(_build_nc())
    nc = _NC_CACHE[0]

    w_pack, vec_pack, a_s = _host_fold(
        W, b, gamma, beta, running_mean, running_var, act_scale
    )
    x_hi = np.ascontiguousarray(x, dtype=np.float32).astype(ml_dtypes.bfloat16)

    in_maps = []
    for c in range(N_CORES):
        sl = slice(c * B_SH, (c + 1) * B_SH)
        in_maps.append({"xh_s": x_hi[sl], "w_all": w_pack, "vec_all": vec_pack})

    trace = bool(os.environ.get("KERNEL_TRACE"))
    try:
        res = run_bass_kernel_spmd(
            nc, in_maps, core_ids=list(range(N_CORES)), trace=trace
        )
    except Exception:
        if not trace:
            raise
        # trace path unavailable (e.g. NTFF hook missing) — run untraced
        res = run_bass_kernel_spmd(
            nc, in_maps, core_ids=list(range(N_CORES)), trace=False
        )
    LAST_RESULTS = res
    codes = np.concatenate([r["y_s"] for r in res.results], axis=0)
    # dequantize the u8 codes during unshard: y = codes * act_scale
    lut = (np.arange(256, dtype=np.float32) * a_s).astype(np.float32)
    return lut[codes]


# revision 23
# speedup vs baseline: 1.0235x; 1.0081x over previous
"""Trainium2 Bass kernel for quantized ConvBNReLU1D (pointwise conv k=1).

Reference computation (see problem spec):
    wq  = fake_quant_int8(W)  (per-tensor power-of-two scale)
    bq  = fake_quant_int8(b)
    y   = wq @ x + bq                  # [Cout,Cin] x [B,Cin,N]
    y   = y * inv + (beta - mean*inv)  # BN inference, inv = gamma*rsqrt(var+eps)
    y   = clip(round(relu(y)/as), 0, 255) * as   # QuantReLU

Strategy (v3 — minimize HBM bytes, then pack the DMA stream):
  - Data-parallel over batch: 32 batches -> 4 per core on 8 cores.
  - Host precomputes the per-channel constants (wq/bq fake-quant is
    bitwise-identical to the fp32 reference; BN+act_scale folded) so the
    device epilogue is one ScalarE ACTIVATE per tile.
  - x is sent as plain bf16 (half the bytes of fp32). wq is exactly
    representable in bf16 (8-bit integer x power of two), so the only
    error is bf16 rounding of x: measured rel err 0.0039 (max one quant
    step), same as the baseline's fp32-split pipeline.
  - Output goes to HBM as u8 quantization codes (the result has only 256
    distinct values: u8 * act_scale); dequant happens on host during
    unshard. 1 byte/elem instead of 4.
  - v4 pipeline fixes over v2 (63 us measured):
      * all constants packed into TWO front-loaded DMAs at the head of
        the scalar ring (v2's 8 strided const loads issued so slowly the
        first matmul waited until +17 us);
      * x buffered at full depth (all 8 tiles) and output tiles 4-deep:
        no WAR stalls in the DMA stream (v2 stalled ~4 us mid-run);
      * transfers stay full-width [128, 4096] — v3 measured that halving
        them costs ~30% DMA efficiency (strided vs contiguous streams).
  - DMA per core: in 8.4 MB (bf16) + out 4.2 MB (u8) = 12.6 MB ~= 35 us
    at the ~358 GB/s HBM/core roofline; TensorE ~28 us; ScalarE ~32 us;
    VectorE/GpSimd unused. ~5-8 us fixed runtime preamble on top.
"""

import os
import sys

import numpy as np

for _p in ("/opt/trn_rl_repo", "/root/.axon_site/_ro/trn_rl_repo"):
    if os.path.isdir(_p) and _p not in sys.path:
        sys.path.insert(0, _p)

from contextlib import ExitStack

import ml_dtypes

import concourse.bacc as bacc
import concourse.tile as tile
from concourse import mybir
from concourse.bass import ts
from concourse.bass_utils import run_bass_kernel_spmd

F32 = mybir.dt.float32
BF16 = mybir.dt.bfloat16
U8 = mybir.dt.uint8
AF = mybir.ActivationFunctionType
ALU = mybir.AluOpType

N_CORES = 8
B, CIN, COUT, N = 32, 256, 256, 4096
B_SH = B // N_CORES  # batches per core
NTILE = 512          # matmul free dim (one fp32 PSUM bank)
EP_BANKS = 4         # PSUM banks per epilogue tile (ACT width = 512*EP_BANKS)
EPW = NTILE * EP_BANKS
NEP = N // EPW       # epilogue tiles per row block (= x half-tiles)
KC = CIN // 128      # K chunks
MC = COUT // 128     # output-channel chunks

QMAX_W = 127.0
BN_EPS = 1e-5

_NC_CACHE = []
LAST_RESULTS = None  # BassKernelResults of the last run (for profiling)


def _build_nc():
    nc = bacc.Bacc("TRN2", target_bir_lowering=False)
    xh_s = nc.declare_dram_parameter("xh_s", [B_SH, CIN, N], BF16, isOutput=False)
    # all 4 lhsT chunks packed side by side: col block k*MC+mo is
    # wT[k*128:(k+1)*128, mo*128:(mo+1)*128]
    w_all = nc.declare_dram_parameter("w_all", [128, KC * MC * 128], BF16, isOutput=False)
    # per-channel vectors packed: col mo = sv chunk, col MC+mo = bv chunk
    vec_all = nc.declare_dram_parameter("vec_all", [128, 2 * MC], F32, isOutput=False)
    y_s = nc.declare_dram_parameter("y_s", [B_SH, COUT, N], U8, isOutput=True)

    with ExitStack() as ctx:
        tc = ctx.enter_context(tile.TileContext(nc))
        consts = ctx.enter_context(tc.tile_pool(name="consts", bufs=1))
        xpool = ctx.enter_context(tc.tile_pool(name="xpool", bufs=B_SH * KC))
        opool = ctx.enter_context(tc.tile_pool(name="opool", bufs=6))
        tmppool = ctx.enter_context(tc.tile_pool(name="tmppool", bufs=2))
        pspool = ctx.enter_context(
            tc.tile_pool(name="pspool", bufs=8 // EP_BANKS, space="PSUM")
        )

        # The schedule is limited by HWDGE descriptor generation (~25 ns
        # per descriptor = one per partition line, serial per ring) and by
        # first-transfer latency (~4.5 us issue-to-land for a 1 MB tile).
        # Only the two HWDGE rings complete transfers promptly (gpsimd's
        # SWDGE completions measured ~7 us lazy). Ring assignment by
        # latency-criticality:
        #   sync:   x loads for batches 0-2 (x0k0 in the very first slot);
        #   scalar: w (warmups + all matmuls key on it), vec, batch-3 x,
        #           and the final stores;
        #   gpsimd: all non-final stores (latency-tolerant, keeps their
        #           descriptor generation off both HWDGE rings).
        w_sb = consts.tile([128, KC * MC * 128], BF16, tag="w")
        nc.scalar.dma_start(out=w_sb, in_=w_all[:, :])
        vec_sb = consts.tile([128, 2 * MC], F32, tag="vec")
        nc.scalar.dma_start(out=vec_sb, in_=vec_all[:, :])

        # Warm the PE clock (HAM un-throttles only after sustained matmul
        # activity; cold-start matmuls run at 1/2-1/4 rate): dummy matmuls
        # off the weight tile, into a PSUM tile that cycles back into the
        # real rotation, keep the array busy while the first x tiles are
        # still in flight.
        warm = pspool.tile([128, EPW], F32, tag="ps")
        for _ in range(12):
            nc.tensor.matmul(
                warm[:, ts(0, NTILE)], lhsT=w_sb[:, ts(0, 128)],
                rhs=w_sb[:, :NTILE], start=True, stop=True,
            )

        ep_i = 0  # global epilogue tile index (Scalar/Vector split)
        for b in range(B_SH):
            # Full [128, 4096] x tiles: 8 KB contiguous per partition —
            # splitting transfers (free-dim or partition-dim) measurably
            # costs more in DMA efficiency than the earlier ramp it buys.
            xh_k = []
            for k in range(KC):
                t = xpool.tile([128, N], BF16, tag="x")
                # b1k1 and batch-3 go on the scalar ring: the sync ring's
                # serial descriptor generation can't deliver b1k1 before
                # the matmuls need it (measured 2.5 us stall), while the
                # scalar ring is free right after w+vec.
                ring = nc.scalar if (b == B_SH - 1 or (b == 1 and k == 1)) else nc.sync
                ring.dma_start(out=t, in_=xh_s[b, k * 128 : (k + 1) * 128, :])
                xh_k.append(t)
            for mo in range(MC):
                ot = opool.tile([128, N], U8, tag="o")
                for ne in range(NEP):
                    ps = pspool.tile([128, EPW], F32, tag="ps")
                    # k-major: x's k1 tile is not needed until four matmuls
                    # after k0, giving its in-flight load extra slack
                    for k in range(KC):
                        for sb in range(EP_BANKS):
                            nt = ne * EP_BANKS + sb
                            nc.tensor.matmul(
                                ps[:, ts(sb, NTILE)],
                                lhsT=w_sb[:, ts(k * MC + mo, 128)],
                                rhs=xh_k[k][:, ts(nt, NTILE)],
                                start=(k == 0), stop=(k == KC - 1),
                            )
                    # Epilogue u8 = clip(round(psum*sv + bv), 0, 255).
                    # ScalarE does 11 tiles (one ACTIVATE each: relu+affine+
                    # sat-u8 convert, exact round-half-even); the otherwise-
                    # idle VectorE takes 5 (affine into f32 tmp — PSUM reads
                    # are 1x on DVE — then clamp to [0,255] and convert;
                    # clamp-first == round-first here, and safe even if the
                    # DVE u8 convert wrapped; measured bit-compatible to
                    # 3 elements in 33.5M). Neither engine paces Tensor.
                    if ep_i % 3 != 1:
                        nc.scalar.activation(
                            ot[:, ts(ne, EPW)], ps, AF.Relu,
                            bias=vec_sb[:, MC + mo : MC + mo + 1],
                            scale=vec_sb[:, mo : mo + 1],
                        )
                    else:
                        tmp = tmppool.tile([128, EPW], F32, tag="tmp")
                        nc.vector.tensor_scalar(
                            tmp, ps,
                            vec_sb[:, mo : mo + 1],
                            vec_sb[:, MC + mo : MC + mo + 1],
                            ALU.mult, ALU.add,
                        )
                        nc.vector.tensor_scalar(
                            ot[:, ts(ne, EPW)], tmp, 255.0, 0.0,
                            ALU.min, ALU.max,
                        )
                    ep_i += 1
                if b == B_SH - 1 and mo == MC - 1:
                    # Last tile: store as two halves on the two HWDGE rings
                    # (both idle by now) so the post-compute tail pays one
                    # parallel descriptor-gen plus a half-size transfer.
                    nc.sync.dma_start(
                        out=y_s[b, mo * 128 : (mo + 1) * 128, : N // 2],
                        in_=ot[:, : N // 2],
                    )
                    nc.scalar.dma_start(
                        out=y_s[b, mo * 128 : (mo + 1) * 128, N // 2 :],
                        in_=ot[:, N // 2 :],
                    )
                else:
                    # gpsimd-ring stores measurably leave HBM idle (lazy
                    # SWDGE completions); the scalar HWDGE ring keeps up
                    # with the epilogue pace.
                    nc.scalar.dma_start(
                        out=y_s[b, mo * 128 : (mo + 1) * 128, :], in_=ot
                    )
    nc.compile()
    return nc


def _host_fold(W, b, gamma, beta, running_mean, running_var, act_scale):
    """Fake-quant W/b exactly as the fp32 reference, fold BN + act scale."""
    f32 = np.float32

    def po2_scale(t):
        maxabs = np.maximum(np.max(np.abs(t)), f32(1e-12)).astype(f32)
        # log2/ceil/exp2 of an f32 value; result is an exact power of two.
        return np.exp2(np.ceil(np.log2(maxabs / f32(QMAX_W)))).astype(f32)

    def fake_quant(t, s):
        return (np.clip(np.round(t / s), -128.0, 127.0) * s).astype(f32)

    wq = fake_quant(W.astype(f32), po2_scale(W.astype(f32)))
    bq = fake_quant(b.astype(f32), po2_scale(b.astype(f32)))
    inv = (gamma.astype(f32) / np.sqrt(running_var.astype(f32) + f32(BN_EPS))).astype(f32)
    shift = (beta.astype(f32) - running_mean.astype(f32) * inv).astype(f32)
    a_s = f32(act_scale)
    sv = (inv / a_s).astype(f32)                    # per-channel matmul scale
    bv = ((bq * inv + shift) / a_s).astype(f32)     # per-channel bias
    # wq is an 8-bit integer times a power of two -> exact in bf16
    wT = np.ascontiguousarray(wq.T)                 # [Cin, Cout] f32
    w_pack = np.empty((128, KC * MC * 128), dtype=ml_dtypes.bfloat16)
    for k in range(KC):
        for mo in range(MC):
            j = (k * MC + mo) * 128
            w_pack[:, j : j + 128] = wT[
                k * 128 : (k + 1) * 128, mo * 128 : (mo + 1) * 128
            ].astype(ml_dtypes.bfloat16)
    w_pack = np.ascontiguousarray(w_pack)
    vec_pack = np.empty((128, 2 * MC), dtype=np.float32)
    for mo in range(MC):
        vec_pack[:, mo] = sv[mo * 128 : (mo + 1) * 128]
        vec_pack[:, MC + mo] = bv[mo * 128 : (mo + 1) * 128]
    return w_pack, vec_pack, a_s


def kernel(x, W, b, gamma, beta, running_mean, running_var, act_scale):
    global LAST_RESULTS
    if not _NC_CACHE:
        _NC_CACHE.append(_build_nc())
    nc = _NC_CACHE[0]

    w_pack, vec_pack, a_s = _host_fold(
        W, b, gamma, beta, running_mean, running_var, act_scale
    )
    x_hi = np.ascontiguousarray(x, dtype=np.float32).astype(ml_dtypes.bfloat16)

    in_maps = []
    for c in range(N_CORES):
        sl = slice(c * B_SH, (c + 1) * B_SH)
        in_maps.append({"xh_s": x_hi[sl], "w_all": w_pack, "vec_all": vec_pack})

    trace = bool(os.environ.get("KERNEL_TRACE"))
    try:
        res = run_bass_kernel_spmd(
            nc, in_maps, core_ids=list(range(N_CORES)), trace=trace
        )
    except Exception:
        if not trace:
            raise
        # trace path unavailable (e.g. NTFF hook missing) — run untraced
        res = run_bass_kernel_spmd(
            nc, in_maps, core_ids=list(range(N_CORES)), trace=False
        )
    LAST_RESULTS = res
    codes = np.concatenate([r["y_s"] for r in res.results], axis=0)
    # dequantize the u8 codes during unshard: y = codes * act_scale
    lut = (np.arange(256, dtype=np.float32) * a_s).astype(np.float32)
    return lut[codes]


# revision 24
# speedup vs baseline: 1.1198x; 1.0940x over previous
"""Trainium2 Bass kernel for quantized ConvBNReLU1D (pointwise conv k=1).

Reference computation (see problem spec):
    wq  = fake_quant_int8(W)  (per-tensor power-of-two scale)
    bq  = fake_quant_int8(b)
    y   = wq @ x + bq                  # [Cout,Cin] x [B,Cin,N]
    y   = y * inv + (beta - mean*inv)  # BN inference, inv = gamma*rsqrt(var+eps)
    y   = clip(round(relu(y)/as), 0, 255) * as   # QuantReLU

Strategy (minimize HBM bytes, then pack the DMA stream):
  - Data-parallel over batch: 32 batches -> 4 per core on 8 cores.
  - Host precomputes the tiny per-channel constants: wq/bq fake-quant
    (bitwise-identical to the fp32 reference) and the BN fold, so the
    device epilogue is a single affine + relu + round + clip per tile.
  - x is sent as plain bf16 (half the bytes of fp32). wq is exactly
    representable in bf16 (8-bit integer x power of two), so the only
    error is bf16 rounding of x: measured rel err 0.0039 (max one quant
    step), identical to the baseline fp32-split pipeline's own rounding
    jitter.
  - The output is written to HBM as u8 *quantization codes* (the result
    has only 256 distinct values: u8 * act_scale); the dequant multiply
    happens on host during unshard. 1 byte/elem instead of 4.
  - Device per [128, 2048] PSUM tile (4 banks):
      TensorE:  8 accumulating bf16 matmuls (K = 2 chunks of 128,
                4 bank-aligned 512-wide slices)
      ScalarE:  u8 = sat_u8(relu(psum*scale_c + bias_c)) — one ACTIVATE;
                the f32->u8 conversion is exact RNE + clamp to [0,255]
                in HW (verified bit-exact vs np.round half-to-even).
  - Pipeline packing (measured constraints: ~25 ns/descriptor HWDGE
    generation serial per ring, one descriptor per partition line; ~4.5 us
    issue-to-land for a 1 MB tile; gpsimd SWDGE completions ~7 us lazy):
      * all constants packed into TWO front-loaded DMAs at the head of
        the scalar ring (8 separate const loads would serialize ~9 us of
        descriptor generation before the first matmul);
      * x buffered at full depth (all 8 tiles), output tiles 4-deep:
        no WAR stalls in the DMA stream;
      * transfers stay full-width [128, 4096] — splitting them (free-dim
        or partition-dim halves) measurably costs more DMA efficiency
        than the earlier ramp it buys.
  - DMA per core: in 8.4 MB (bf16) + out 4.2 MB (u8) = 12.6 MB ~= 35 us
    at the ~358 GB/s HBM/core roofline (vs 99 us for the f32-in/f32-out
    pipeline). TensorE ~31 us, ScalarE ~33 us, both under the DMA
    roofline; measured 58.3 us end-to-end including the ~7 us fixed
    runtime preamble and ramp/drain edges.
"""

import os
import sys

import numpy as np

for _p in ("/opt/trn_rl_repo", "/root/.axon_site/_ro/trn_rl_repo"):
    if os.path.isdir(_p) and _p not in sys.path:
        sys.path.insert(0, _p)

from contextlib import ExitStack

import ml_dtypes

import concourse.bacc as bacc
import concourse.tile as tile
from concourse import mybir
from concourse.bass import ts
from concourse.bass_utils import run_bass_kernel_spmd

F32 = mybir.dt.float32
BF16 = mybir.dt.bfloat16
U8 = mybir.dt.uint8
AF = mybir.ActivationFunctionType

N_CORES = 8
B, CIN, COUT, N = 32, 256, 256, 4096
B_SH = B // N_CORES  # batches per core
NTILE = 512          # matmul free dim (one fp32 PSUM bank)
EP_BANKS = 4         # PSUM banks per epilogue tile (ACT width = 512*EP_BANKS)
EPW = NTILE * EP_BANKS
NEP = N // EPW       # epilogue tiles per row block
KC = CIN // 128      # K chunks
MC = COUT // 128     # output-channel chunks

QMAX_W = 127.0
BN_EPS = 1e-5

_NC_CACHE = []
LAST_RESULTS = None  # BassKernelResults of the last run (for profiling)


def _build_nc():
    nc = bacc.Bacc("TRN2", target_bir_lowering=False)
    xh_s = nc.declare_dram_parameter("xh_s", [B_SH, CIN, N], BF16, isOutput=False)
    # all 4 lhsT chunks packed side by side: col block k*MC+mo is
    # wT[k*128:(k+1)*128, mo*128:(mo+1)*128]
    w_all = nc.declare_dram_parameter("w_all", [128, KC * MC * 128], BF16, isOutput=False)
    # per-channel vectors packed: col mo = sv chunk, col MC+mo = bv chunk
    vec_all = nc.declare_dram_parameter("vec_all", [128, 2 * MC], F32, isOutput=False)
    y_s = nc.declare_dram_parameter("y_s", [B_SH, COUT, N], U8, isOutput=True)

    with ExitStack() as ctx:
        tc = ctx.enter_context(tile.TileContext(nc))
        consts = ctx.enter_context(tc.tile_pool(name="consts", bufs=1))
        xpool = ctx.enter_context(tc.tile_pool(name="xpool", bufs=B_SH * KC))
        opool = ctx.enter_context(tc.tile_pool(name="opool", bufs=4))
        pspool = ctx.enter_context(
            tc.tile_pool(name="pspool", bufs=8 // EP_BANKS, space="PSUM")
        )

        # Packed constants, front-loaded on the (otherwise idle-until-stores)
        # scalar ring: two DMAs instead of eight, landing before the first
        # full x tile finishes on the sync ring, so the first matmul is
        # never weight-gated and the x load stream is never queued behind
        # the descriptor-heavy small transfers.
        w_sb = consts.tile([128, KC * MC * 128], BF16, tag="w")
        nc.scalar.dma_start(out=w_sb, in_=w_all[:, :])
        vec_sb = consts.tile([128, 2 * MC], F32, tag="vec")
        nc.scalar.dma_start(out=vec_sb, in_=vec_all[:, :])

        for b in range(B_SH):
            # Full [128, 4096] x tiles: 8 KB contiguous per partition —
            # splitting these (or the stores) into halves costs ~30% DMA
            # efficiency (measured), far more than the earlier ramp it buys.
            xh_k = []
            for k in range(KC):
                t = xpool.tile([128, N], BF16, tag="x")
                nc.sync.dma_start(out=t, in_=xh_s[b, k * 128 : (k + 1) * 128, :])
                xh_k.append(t)
            for mo in range(MC):
                ot = opool.tile([128, N], U8, tag="o")
                for ne in range(NEP):
                    ps = pspool.tile([128, EPW], F32, tag="ps")
                    for sb in range(EP_BANKS):
                        nt = ne * EP_BANKS + sb
                        pslice = ps[:, ts(sb, NTILE)]
                        nc.tensor.matmul(
                            pslice, lhsT=w_sb[:, ts(mo, 128)],
                            rhs=xh_k[0][:, ts(nt, NTILE)],
                            start=True, stop=False,
                        )
                        nc.tensor.matmul(
                            pslice, lhsT=w_sb[:, ts(MC + mo, 128)],
                            rhs=xh_k[1][:, ts(nt, NTILE)],
                            start=False, stop=True,
                        )
                    # u8 = sat_u8(relu(psum*sv + bv)): the f32->u8 convert
                    # is exact round-half-even + clamp to [0,255] in HW.
                    nc.scalar.activation(
                        ot[:, ts(ne, EPW)], ps, AF.Relu,
                        bias=vec_sb[:, MC + mo : MC + mo + 1],
                        scale=vec_sb[:, mo : mo + 1],
                    )
                nc.scalar.dma_start(
                    out=y_s[b, mo * 128 : (mo + 1) * 128, :], in_=ot
                )
    nc.compile()
    return nc


def _host_fold(W, b, gamma, beta, running_mean, running_var, act_scale):
    """Fake-quant W/b exactly as the fp32 reference, fold BN + act scale."""
    f32 = np.float32

    def po2_scale(t):
        maxabs = np.maximum(np.max(np.abs(t)), f32(1e-12)).astype(f32)
        # log2/ceil/exp2 of an f32 value; result is an exact power of two.
        return np.exp2(np.ceil(np.log2(maxabs / f32(QMAX_W)))).astype(f32)

    def fake_quant(t, s):
        return (np.clip(np.round(t / s), -128.0, 127.0) * s).astype(f32)

    wq = fake_quant(W.astype(f32), po2_scale(W.astype(f32)))
    bq = fake_quant(b.astype(f32), po2_scale(b.astype(f32)))
    inv = (gamma.astype(f32) / np.sqrt(running_var.astype(f32) + f32(BN_EPS))).astype(f32)
    shift = (beta.astype(f32) - running_mean.astype(f32) * inv).astype(f32)
    a_s = f32(act_scale)
    sv = (inv / a_s).astype(f32)                    # per-channel matmul scale
    bv = ((bq * inv + shift) / a_s).astype(f32)     # per-channel bias
    # wq is an 8-bit integer times a power of two -> exact in bf16
    wT = np.ascontiguousarray(wq.T)                 # [Cin, Cout] f32
    w_pack = np.empty((128, KC * MC * 128), dtype=ml_dtypes.bfloat16)
    for k in range(KC):
        for mo in range(MC):
            j = (k * MC + mo) * 128
            w_pack[:, j : j + 128] = wT[
                k * 128 : (k + 1) * 128, mo * 128 : (mo + 1) * 128
            ].astype(ml_dtypes.bfloat16)
    w_pack = np.ascontiguousarray(w_pack)
    vec_pack = np.empty((128, 2 * MC), dtype=np.float32)
    for mo in range(MC):
        vec_pack[:, mo] = sv[mo * 128 : (mo + 1) * 128]
        vec_pack[:, MC + mo] = bv[mo * 128 : (mo + 1) * 128]
    return w_pack, vec_pack, a_s


def kernel(x, W, b, gamma, beta, running_mean, running_var, act_scale):
    global LAST_RESULTS
    if not _NC_CACHE:
        _NC_CACHE.append(_build_nc())
    nc = _NC_CACHE[0]

    w_pack, vec_pack, a_s = _host_fold(
        W, b, gamma, beta, running_mean, running_var, act_scale
    )
    x_hi = np.ascontiguousarray(x, dtype=np.float32).astype(ml_dtypes.bfloat16)

    in_maps = []
    for c in range(N_CORES):
        sl = slice(c * B_SH, (c + 1) * B_SH)
        in_maps.append({"xh_s": x_hi[sl], "w_all": w_pack, "vec_all": vec_pack})

    trace = bool(os.environ.get("KERNEL_TRACE"))
    try:
        res = run_bass_kernel_spmd(
            nc, in_maps, core_ids=list(range(N_CORES)), trace=trace
        )
    except Exception:
        if not trace:
            raise
        # trace path unavailable (e.g. NTFF hook missing) — run untraced
        res = run_bass_kernel_spmd(
            nc, in_maps, core_ids=list(range(N_CORES)), trace=False
        )
    LAST_RESULTS = res
    codes = np.concatenate([r["y_s"] for r in res.results], axis=0)
    # dequantize the u8 codes during unshard: y = codes * act_scale
    lut = (np.arange(256, dtype=np.float32) * a_s).astype(np.float32)
    return lut[codes]
